# revision 19
# baseline (speedup 1.0000x reference)
"""GPT-2 attention block (B=4, S=1024, D=1024, H=16) on 8 TRN2 NeuronCores.

Tensor-parallel over heads: core i holds heads 2i, 2i+1. qkv is computed
with per-core weight columns in transposed layout [cols, tokens]; v is
PE-transposed into [tokens, cols] stationary tiles. Attention scores are
built directly in transposed layout P^T[k, q] so they feed the AV matmul
as the moving operand with no extra data movement; the softmax
denominator rides along the AV matmul as an appended ones-column block
of the stationary operand. The causal mask is folded into the PE: a
constant -1e4 upper-triangle matrix is accumulated onto the diagonal
128x128 score block via matmul(lhsT=I, rhs=maskM, start=False), so the
per-tile chain is PE -> ACT(exp) -> PE with no vector-engine hop.
Softmax division uses the ACT-engine reciprocal. Four per-batch
AllToAlls over interleaved 64-token blocks convert head-sharding to
token-sharding (the first hides the inter-core arrival skew under the
remaining attention compute; per-batch c_proj keeps the PE warm through
the tail); c_proj needs no reduction: each core emits a disjoint
[D, 512]-token output shard.
"""

import numpy as np
import ml_dtypes

import concourse.bass as bass
import concourse.mybir as mybir
import concourse.tile as tile
from concourse import bacc
from concourse.bass_utils import run_bass_kernel_spmd

B, S, D, H = 4, 1024, 1024, 16
HD = D // H  # 64
NT = B * S  # 4096 tokens
N_CORES = 8
CORE_IDS = list(range(N_CORES))
BF16 = mybir.dt.bfloat16
F32 = mybir.dt.float32
AF = mybir.ActivationFunctionType

_CACHE = {}


def act_reciprocal(nc, out, in_):
    """ACT-engine reciprocal (~1e-5 rel err, plenty for this kernel), emitted
    directly because bass's wrapper bans it for ULP-level accuracy reasons."""
    eng = nc.scalar
    inputs = [
        eng.lower_ap(in_),
        mybir.ImmediateValue(dtype=mybir.dt.float32, value=0.0),
        mybir.ImmediateValue(dtype=mybir.dt.float32, value=1.0),
        mybir.ImmediateValue(dtype=mybir.dt.float32, value=0.0),
    ]
    return eng.add_instruction(
        mybir.InstActivation(
            name=nc.get_next_instruction_name(),
            func=mybir.ActivationFunctionType.Reciprocal,
            ins=inputs,
            outs=[eng.lower_ap(out)],
        )
    )


def build_nc():
    nc = bacc.Bacc("TRN2", target_bir_lowering=False, debug=False, num_devices=N_CORES)

    xt_d = nc.dram_tensor("xt", [D, NT], BF16, kind="ExternalInput")
    wqkv_d = nc.dram_tensor("wqkv", [D, 384], BF16, kind="ExternalInput")
    bqkv_d = nc.dram_tensor("bqkv", [3, 128, 1], F32, kind="ExternalInput")
    wp_d = nc.dram_tensor("wp", [D, D], BF16, kind="ExternalInput")
    eye_d = nc.dram_tensor("eye", [128, 128], BF16, kind="ExternalInput")
    maskm_d = nc.dram_tensor("maskm", [128, 128], BF16, kind="ExternalInput")
    out_d = nc.dram_tensor("out", [D, 512], F32, kind="ExternalOutput")

    with tile.TileContext(nc) as tc:
        with (
            tc.tile_pool(name="persist", bufs=1) as pp,
            tc.tile_pool(name="xin", bufs=3) as xp,
            tc.tile_pool(name="ptp", bufs=8) as ptp,
            tc.tile_pool(name="work", bufs=2) as wk,
            tc.tile_pool(name="ps", bufs=6, space="PSUM") as psp,
            tc.tile_pool(name="ps_at", bufs=1, space="PSUM") as ps_at,
            tc.tile_pool(name="dram", bufs=1, space="DRAM") as dp,
        ):
            # ---- persistent weights / constants (batched DMAs) ----
            wqkv = pp.tile([128, 8, 384], BF16, tag="wqkv")
            nc.sync.dma_start(wqkv[:], wqkv_d.rearrange("(a p) c -> p a c", p=128))
            eye = pp.tile([128, 128], BF16, tag="eye")
            nc.sync.dma_start(eye[:], eye_d[:])
            maskm = pp.tile([128, 128], BF16, tag="maskm")
            nc.sync.dma_start(maskm[:], maskm_d[:])
            bias = []
            for m in range(3):
                t = pp.tile([128, 1], F32, tag=f"bias{m}", name=f"bias{m}")
                nc.sync.dma_start(t[:], bqkv_d[m])
                bias.append(t)
            # wp is only needed by c_proj; keep it off the sync queue's head
            wp_sb = pp.tile([128, 8, 1024], BF16, tag="wp")
            wpsrc = wp_d.rearrange("(a p) c -> p a c", p=128)
            for g in range(4):
                nc.scalar.dma_start(
                    wp_sb[:, 2 * g : 2 * (g + 1), :], wpsrc[:, 2 * g : 2 * (g + 1), :]
                )

            # per-batch buffers for b0-b2 ([1024,128]); per-span for b3 ([1024,64])
            a2a_in = [
                dp.tile([1024, 128 if p < 3 else 64], BF16, name=f"a2ain{p}")
                for p in range(5)
            ]
            a2a_out = [
                dp.tile([1024, 128 if p < 3 else 64], BF16, name=f"a2aout{p}")
                for p in range(5)
            ]

            qt, kt, vt = [], [], []
            vaug = {}
            at_sb = []

            def qkv_chunk(t):
                xb = xp.tile([128, 8, 512], BF16, tag="x", name=f"x_{t}")
                xsrc = xt_d[:, 512 * t : 512 * (t + 1)].rearrange(
                    "(a p) c -> p a c", p=128
                )
                for g in range(4):
                    nc.sync.dma_start(
                        xb[:, 2 * g : 2 * (g + 1), :], xsrc[:, 2 * g : 2 * (g + 1), :]
                    )
                for m, store in enumerate((qt, kt, vt)):
                    ps = psp.tile([128, 512], F32, tag="ps", name=f"qkv{m}_{t}")
                    for k in range(8):
                        nc.tensor.matmul(
                            ps[:],
                            wqkv[:, k, 128 * m : 128 * (m + 1)],
                            xb[:, k, :],
                            start=(k == 0),
                            stop=(k == 7),
                        )
                    sb = pp.tile([128, 512], BF16, tag=f"qkv{m}_{t}", name=f"qkv{m}_{t}")
                    nc.vector.tensor_scalar_add(sb[:], ps[:], bias[m][:])
                    store.append(sb)
                # v_aug: [tokens, (v_h0 | ones | v_h1 | ones)] via PE transpose
                for i in range(4):
                    va = pp.tile([128, 256], BF16, tag=f"va{t}_{i}", name=f"va{t}_{i}")
                    va4 = va.rearrange("p (a b) -> p a b", b=64)
                    tp = psp.tile([128, 128], BF16, tag="ps", name=f"vt{t}_{i}")
                    nc.tensor.transpose(
                        tp[:], vt[t][:, 128 * i : 128 * (i + 1)], eye[:]
                    )
                    nc.vector.tensor_copy(
                        va4[:, 0:3:2, :], tp.rearrange("p (a b) -> p a b", b=64)
                    )
                    nc.gpsimd.memset(va4[:, 1:4:2, :], 1.0)
                    vaug[(t, i)] = va

            def attention_span(b, s):
                aT = at_sb[b]
                tcq = 2 * b + s
                last = 4 * s + 3
                at_ps = [
                    ps_at.tile([128, 512], F32, tag=f"at{h}", name=f"at{h}_{b}_{s}")
                    for h in range(2)
                ]
                for kc in range(last + 1):
                    off = max(0, kc * 128 - s * 512)
                    width = 512 - off
                    tck = 2 * b + kc // 4
                    kcol = (kc % 4) * 128
                    dq = kc * 128 - s * 512  # diag col in span coords
                    va = vaug[(tck, kc % 4)]
                    for h in range(2):
                        pt_ps = psp.tile(
                            [128, 512], F32, tag="ps", name=f"pt{b}_{s}_{kc}_{h}"
                        )
                        nc.tensor.matmul(
                            pt_ps[:, 0:width],
                            kt[tck][64 * h : 64 * h + 64, kcol : kcol + 128],
                            qt[tcq][64 * h : 64 * h + 64, off:512],
                            start=True,
                            stop=(dq < 0),
                        )
                        if dq >= 0:
                            dcol = dq - off
                            nc.tensor.matmul(
                                pt_ps[:, dcol : dcol + 128],
                                eye[:],
                                maskm[:],
                                start=False,
                                stop=True,
                            )
                        pt_sb = ptp.tile(
                            [128, 512], BF16, tag="pt", name=f"ptsb{b}_{s}_{kc}_{h}"
                        )
                        nc.scalar.activation(
                            pt_sb[:, 0:width], pt_ps[:, 0:width], AF.Exp
                        )
                        # stationary: h -> [v_h | ones], both contiguous
                        nc.tensor.matmul(
                            at_ps[h][:, off:512],
                            va[:, 128 * h : 128 * (h + 1)],
                            pt_sb[:, 0:width],
                            start=(kc == 0),
                            stop=(kc == last),
                        )
                for h in range(2):
                    rec = wk.tile([64, 512], F32, tag=f"rec{h}", name=f"rec{h}_{b}_{s}")
                    act_reciprocal(nc, rec[:], at_ps[h][64:128, :])
                    nc.vector.tensor_mul(
                        aT[64 * h : 64 * h + 64, 512 * s : 512 * (s + 1)],
                        at_ps[h][0:64, :],
                        rec[:],
                    )

            def a2a_batch(b):
                # batch b: token blocks g=16b+l; dest core j=g%8, slot m=g//8
                # (m = 2b, 2b+1). One strided DMA per dest core.
                aT3 = at_sb[b].rearrange("p (a b) -> p a b", b=64)
                for j in range(8):
                    dst = a2a_in[b][128 * j : 128 * (j + 1), :]
                    nc.sync.dma_start(
                        dst.rearrange("p (a b) -> p a b", b=64),
                        aT3[:, j : j + 9 : 8, :],
                    )
                nc.gpsimd.collective_compute(
                    "AllToAll",
                    mybir.AluOpType.bypass,
                    replica_groups=[CORE_IDS],
                    ins=[a2a_in[b].opt()],
                    outs=[a2a_out[b].opt()],
                )

            def a2a_half(s):
                # batch 3, span s: blocks l = 8s..8s+8, slot m = 6+s
                p = 3 + s
                for j in range(8):
                    nc.sync.dma_start(
                        a2a_in[p][128 * j : 128 * (j + 1), :],
                        at_sb[3][:, 512 * s + 64 * j : 512 * s + 64 * (j + 1)],
                    )
                nc.gpsimd.collective_compute(
                    "AllToAll",
                    mybir.AluOpType.bypass,
                    replica_groups=[CORE_IDS],
                    ins=[a2a_in[p].opt()],
                    outs=[a2a_out[p].opt()],
                )

            def cproj_part(p, col0, width):
                ae = pp.tile([128, 8, width], BF16, tag=f"ae{p}", name=f"ae{p}")
                nc.sync.dma_start(ae[:], a2a_out[p].rearrange("(a p) c -> p a c", p=128))
                for m in range(8):
                    ps = psp.tile([128, width], F32, tag="ps", name=f"cp{p}_{m}")
                    for k in range(8):
                        nc.tensor.matmul(
                            ps[:],
                            wp_sb[:, k, 128 * m : 128 * (m + 1)],
                            ae[:, k, :],
                            start=(k == 0),
                            stop=(k == 7),
                        )
                    osb = wk.tile([128, width], F32, tag="osb", name=f"osb{p}_{m}")
                    nc.vector.tensor_copy(osb[:], ps[:])
                    nc.sync.dma_start(
                        out_d[128 * m : 128 * (m + 1), col0 : col0 + width], osb[:]
                    )

            # ---- program ----
            for b in range(B):
                aT = pp.tile([128, 1024], BF16, tag=f"aT{b}", name=f"aT{b}")
                at_sb.append(aT)
                qkv_chunk(2 * b)
                attention_span(b, 0)
                if b == 3:
                    a2a_half(0)
                qkv_chunk(2 * b + 1)
                attention_span(b, 1)
                if b < 3:
                    a2a_batch(b)
                else:
                    a2a_half(1)
                if 1 <= b <= 2:
                    cproj_part(b - 1, 128 * (b - 1), 128)
            cproj_part(2, 256, 128)
            cproj_part(3, 384, 64)
            cproj_part(4, 448, 64)

    nc.compile()
    return nc


def _prep_inputs(x, w_attn, b_attn, w_proj):
    bf = ml_dtypes.bfloat16
    xt = np.ascontiguousarray(x.reshape(NT, D).T).astype(bf)
    scale = 1.0 / np.sqrt(np.float32(HD))
    wp = w_proj.astype(bf)
    r, c = np.arange(128)[:, None], np.arange(128)[None, :]
    eye = np.eye(128, dtype=np.float32).astype(bf)
    maskm = np.where(r <= c, 0.0, -10000.0).astype(np.float32).astype(bf)
    in_maps = []
    for i in range(N_CORES):
        cc = 128 * i
        wq = (w_attn[:, cc : cc + 128] * scale).astype(bf)
        wkk = w_attn[:, D + cc : D + cc + 128].astype(bf)
        wv = w_attn[:, 2 * D + cc : 2 * D + cc + 128].astype(bf)
        wqkv = np.concatenate([wq, wkk, wv], axis=1)
        bqkv = np.stack(
            [
                (b_attn[cc : cc + 128] * scale).astype(np.float32),
                b_attn[D + cc : D + cc + 128].astype(np.float32),
                b_attn[2 * D + cc : 2 * D + cc + 128].astype(np.float32),
            ]
        ).reshape(3, 128, 1)
        in_maps.append(
            {"xt": xt, "wqkv": wqkv, "bqkv": bqkv, "wp": wp, "eye": eye, "maskm": maskm}
        )
    return in_maps


def run_on_hw(in_maps, trace=False, **kw):
    if "nc" not in _CACHE:
        _CACHE["nc"] = build_nc()
    return run_bass_kernel_spmd(_CACHE["nc"], in_maps, CORE_IDS, trace=trace, **kw)


def assemble_output(results, b_proj):
    # core j's out column block m (64 cols) is token block g = j + 8m
    outT = np.empty((D, NT), dtype=np.float32)
    for j in range(N_CORES):
        o = results[j]["out"]
        for m in range(8):
            g = j + 8 * m
            outT[:, 64 * g : 64 * (g + 1)] = o[:, 64 * m : 64 * (m + 1)]
    return (outT.T + b_proj[None, :].astype(np.float32)).reshape(B, S, D)


def kernel(x, w_attn, b_attn, w_proj, b_proj):
    in_maps = _prep_inputs(
        np.asarray(x, dtype=np.float32),
        np.asarray(w_attn, dtype=np.float32),
        np.asarray(b_attn, dtype=np.float32),
        np.asarray(w_proj, dtype=np.float32),
    )
    res = run_on_hw(in_maps)
    return assemble_output(res.results, np.asarray(b_proj, dtype=np.float32))
